# revision 14
# baseline (speedup 1.0000x reference)
"""DGCN dynamic-adjacency kernel for TRN2, data-parallel over batch B=8.

Per core (batch element b):
  h[f]    = mean_L prelu(x[b,f,:])          (phase A: DMA-stream + fused DVE reduce)
  stats   = AllGather h across 8 cores      (phase B: BN batch stats, two-pass var)
  hhat    = (h - mu) * rsqrt(var+eps) * bnw + bnb
  mask    = 1[hhat_n * hhat_m > 0.81] (+I)  (phase C: fused DVE/ACT ops on a
            degree d = row-sum(mask)         partition-broadcast row of hhat)
  c       = rsqrt(d + 1)
  y[n,m]  = mask * c_n * c_m                (phase E: one fused DVE op per chunk)
The output PReLU is the identity since y >= 0 everywhere.  No TensorE use at
all: K=1 outer-product matmuls are ~1us per [128,512] on TRN2, far slower than
streaming the same elements through DVE/ACT with per-partition scalars.
"""

import numpy as np

import concourse.bacc as bacc
import concourse.mybir as mybir
import concourse.tile as tile
from concourse.bass_utils import run_bass_kernel_spmd

B, F, L, P = 8, 2048, 1024, 128
NJ = F // P  # 16 row chunks
THRESH = 0.81
BN_EPS = 1e-5
f32 = mybir.dt.float32
bf16 = mybir.dt.bfloat16

# chunks whose mask is computed on the scalar engine (sign+relu, 2 passes)
# vs the vector engine (fused mult+is_gt, 1 pass); ACT pass is ~1.25x DVE rate.
# The second half carries more ACT chunks because DVE also runs the top-left
# output blocks concurrently (staircase overlap).
ACT_CHUNKS = {1, 4, 6, 8, 10, 12, 14}
NH = NJ // 2  # 8: chunks per staircase half
FH = F // 2  # 1024: columns per half

_NC_CACHE: dict = {}


def _build(w1: float, num_devices: int = B):
    nc = bacc.Bacc(
        "TRN2", target_bir_lowering=False, debug=False, num_devices=num_devices
    )
    x = nc.declare_dram_parameter("x", [F, L], f32, isOutput=False)
    bnw = nc.declare_dram_parameter("bn_weight", [F], f32, isOutput=False)
    bnb = nc.declare_dram_parameter("bn_bias", [F], f32, isOutput=False)
    iden = nc.declare_dram_parameter("iden", [P, P], bf16, isOutput=False)
    y = nc.declare_dram_parameter("y", [F, F], f32, isOutput=True)

    AX = mybir.AxisListType.X
    OP = mybir.AluOpType
    ACT = mybir.ActivationFunctionType
    # prelu(v, w) = max(w*v, v) for w <= 1, min otherwise
    prelu_op = OP.max if w1 <= 1.0 else OP.min

    with tile.TileContext(nc) as tc:
        with (
            tc.tile_pool(name="dram", bufs=1, space="DRAM") as dpool,
            tc.tile_pool(name="xin", bufs=3) as xpool,
            tc.tile_pool(name="small", bufs=1) as spool,
            tc.tile_pool(name="acc", bufs=4) as apool,
            tc.tile_pool(name="mask", bufs=1) as mpool,
            tc.tile_pool(name="sgn", bufs=2) as gpool,
            tc.tile_pool(name="yout", bufs=3) as ypool,
        ):
            # threshold tile for the fused compare (in1 of scalar_tensor_tensor)
            thr = spool.tile([P, F], f32, tag="thr")
            nc.gpsimd.memset(thr[:], THRESH)
            idt = spool.tile([P, P], bf16, tag="idt")
            nc.sync.dma_start(idt[:], iden[:])
            nthr = spool.tile([P, 1], f32, tag="nthr")
            nc.vector.memset(nthr[:], -THRESH)
            epsb = spool.tile([P, 1], f32, tag="epsb")
            nc.vector.memset(epsb[:], BN_EPS)
            bnw_t = spool.tile([P, NJ], f32, tag="bnw_t")
            nc.sync.dma_start(bnw_t[:], bnw[:].rearrange("(j p) -> p j", p=P))
            bnb_t = spool.tile([P, NJ], f32, tag="bnb_t")
            nc.sync.dma_start(bnb_t[:], bnb[:].rearrange("(j p) -> p j", p=P))

            # ---------- phase A: h[f] = mean_L prelu(x[f, :]) ----------
            hsb = spool.tile([P, NJ], f32, tag="hsb")
            for j in range(NJ):
                xt = xpool.tile([P, L], f32, tag="x")
                nc.sync.dma_start(xt[:], x[j * P : (j + 1) * P, :])
                sp = apool.tile([P, 1], f32, tag="sp")
                # xt = prelu(xt) in place, sp = row-sum(prelu(xt))
                nc.vector.scalar_tensor_tensor(
                    xt[:], xt[:], w1, xt[:], op0=OP.mult, op1=prelu_op, accum_out=sp[:]
                )
                nc.vector.tensor_scalar(
                    hsb[:, j : j + 1], sp[:], 1.0 / L, None, op0=OP.mult
                )

            # ---------- phase B: BN batch stats via AllGather ----------
            hd = dpool.tile([F], f32, tag="hd")
            Hd = dpool.tile([B, F], f32, tag="Hd")
            nc.sync.dma_start(hd[:].rearrange("(j p) -> p j", p=P), hsb[:])
            if num_devices > 1:
                nc.gpsimd.collective_compute(
                    "AllGather",
                    OP.bypass,
                    replica_groups=[list(range(B))],
                    ins=[hd[:].opt()],
                    outs=[Hd[:].opt()],
                )
            else:  # single-core timing model variant: fake the gather locally
                for b in range(B):
                    nc.sync.dma_start(
                        Hd[b : b + 1, :], hd[:].rearrange("(o f) -> o f", o=1)
                    )
            # Hp[p, b*NJ + j] = Hd[b, j*128+p] — one balanced 3-dim DMA
            Hp = spool.tile([P, B * NJ], f32, tag="Hp")
            nc.sync.dma_start(
                Hp[:].rearrange("p (b j) -> p b j", j=NJ),
                Hd[:].rearrange("b (j p) -> p b j", p=P),
            )
            Hp3 = Hp[:].rearrange("p (b j) -> p j b", j=NJ)
            smu = spool.tile([P, NJ], f32, tag="smu")
            nc.vector.tensor_reduce(smu[:], Hp3, axis=AX, op=OP.add)
            mu = spool.tile([P, NJ], f32, tag="mu")
            nc.vector.tensor_scalar(mu[:], smu[:], 1.0 / B, None, op0=OP.mult)
            # two-pass variance: hcall = Hp - mu (mu free-dim-broadcast over b)
            mu3b = mu[:].rearrange("p (o j) -> p o j", o=1).to_broadcast([P, B, NJ])
            hcall = spool.tile([P, B * NJ], f32, tag="hcall")
            nc.vector.tensor_sub(
                hcall[:].rearrange("p (b j) -> p b j", j=NJ),
                Hp[:].rearrange("p (b j) -> p b j", j=NJ),
                mu3b,
            )
            hsq = spool.tile([P, B * NJ], f32, tag="hsq")
            nc.vector.tensor_mul(hsq[:], hcall[:], hcall[:])
            ssq = spool.tile([P, NJ], f32, tag="ssq")
            nc.vector.tensor_reduce(
                ssq[:], hsq[:].rearrange("p (b j) -> p j b", j=NJ), axis=AX, op=OP.add
            )
            # std = sqrt(ssq/B + eps): fold the 1/B into the activation scale
            stdt = spool.tile([P, NJ], f32, tag="stdt")
            nc.scalar.activation(
                stdt[:], ssq[:], ACT.Sqrt, bias=epsb[:], scale=1.0 / B
            )
            inv = spool.tile([P, NJ], f32, tag="inv")
            nc.vector.reciprocal(inv[:], stdt[:])
            # hh2 = ((h - mu) * inv) * bnw + bnb, matching the reference's order
            hc = spool.tile([P, NJ], f32, tag="hc")
            nc.vector.tensor_sub(hc[:], hsb[:], mu[:])
            hn = spool.tile([P, NJ], f32, tag="hn")
            nc.vector.tensor_mul(hn[:], hc[:], inv[:])
            hw = spool.tile([P, NJ], f32, tag="hw")
            nc.vector.tensor_mul(hw[:], hn[:], bnw_t[:])
            hh2 = spool.tile([P, NJ], f32, tag="hh2")
            nc.vector.tensor_add(hh2[:], hw[:], bnb_t[:])
            hhd = dpool.tile([F], f32, tag="hhd")
            nc.sync.dma_start(hhd[:].rearrange("(j p) -> p j", p=P), hh2[:])
            # hb[p, m] = hhat[m] for every partition p (broadcast row)
            hb = spool.tile([P, F], f32, tag="hb")
            nc.sync.dma_start(
                hb[:], hhd[:].rearrange("(o f) -> o f", o=1).to_broadcast([P, F])
            )

            # ---------- phases C/D/E, staircase over column halves ----------
            # C(0..7) -> D_left -> {C(8..15) || E[rows 0..7, left cols]}
            #         -> D_right -> E[rows 0..7 right cols; rows 8..15 full]
            masks = []
            dhalf = []  # [dsbL, dsbR]
            chalf = []  # [csbL, csbR]

            def mask_chunk(i, dtile, dcol):
                mt = mpool.tile([P, F], bf16, tag=f"m{i}", name=f"mask{i}")
                if i in ACT_CHUNKS:
                    # sgn = sign(hb*h_n - thresh) in {-1,0,1}; mask = relu(sgn)
                    sg = gpool.tile([P, F], bf16, tag="sg", name=f"sg{i}")
                    nc.scalar.activation(
                        sg[:], hb[:], ACT.Sign, bias=nthr[:], scale=hh2[:, i : i + 1]
                    )
                    nc.scalar.activation(
                        mt[:], sg[:], ACT.Relu, accum_out=dtile[:, dcol : dcol + 1]
                    )
                else:
                    # mask = (hb * h_n) > thr, degree = row-sum(mask)
                    nc.vector.scalar_tensor_tensor(
                        mt[:],
                        hb[:],
                        hh2[:, i : i + 1],
                        thr[:],
                        op0=OP.mult,
                        op1=OP.is_gt,
                        accum_out=dtile[:, dcol : dcol + 1],
                    )
                # add identity on the diagonal block
                nc.vector.tensor_add(
                    mt[:, i * P : (i + 1) * P], mt[:, i * P : (i + 1) * P], idt[:]
                )
                masks.append(mt)

            def d_half(h):
                # c = 1/sqrt(d+1) for one half's row chunks, + broadcast row tile
                sqd = spool.tile([P, NH], f32, tag=f"sqd{h}")
                nc.scalar.activation(sqd[:], dhalf[h][:], ACT.Sqrt, bias=1.0)
                cs = spool.tile([P, NH], f32, tag=f"csb{h}")
                nc.vector.reciprocal(cs[:], sqd[:])
                chalf.append(cs)
                cdh = dpool.tile([FH], f32, tag=f"cd{h}")
                nc.sync.dma_start(cdh[:].rearrange("(j p) -> p j", p=P), cs[:])
                cbth = spool.tile([P, FH], f32, tag=f"cbt{h}")
                nc.sync.dma_start(
                    cbth[:],
                    cdh[:].rearrange("(o f) -> o f", o=1).to_broadcast([P, FH]),
                )
                return cbth

            def out_block(i, h, cbth):
                # y[rows of chunk i, cols of half h] = mask * c_n * c_m
                cs = chalf[i // NH]
                yt = ypool.tile([P, FH], f32, tag="yt", name=f"yt{i}_{h}")
                nc.vector.scalar_tensor_tensor(
                    yt[:],
                    cbth[:],
                    cs[:, i % NH : i % NH + 1],
                    masks[i][:, h * FH : (h + 1) * FH],
                    op0=OP.mult,
                    op1=OP.mult,
                )
                nc.sync.dma_start(
                    y[i * P : (i + 1) * P, h * FH : (h + 1) * FH], yt[:]
                )

            dhalf.append(spool.tile([P, NH], f32, tag="dsbL"))
            dhalf.append(spool.tile([P, NH], f32, tag="dsbR"))
            for i in range(NH):
                mask_chunk(i, dhalf[0], i)
            cbtL = d_half(0)
            for i in range(NH, NJ):
                mask_chunk(i, dhalf[1], i - NH)
                out_block(i - NH, 0, cbtL)  # top-left blocks overlap C's 2nd half
            cbtR = d_half(1)
            for i in range(NH):
                out_block(i, 1, cbtR)
            for i in range(NH, NJ):
                out_block(i, 0, cbtL)
                out_block(i, 1, cbtR)

    nc.compile()
    return nc


def _get_nc(w1: float):
    key = round(w1, 9)
    if key not in _NC_CACHE:
        _NC_CACHE[key] = _build(w1)
    return _NC_CACHE[key]


def _in_maps(x, bn_weight, bn_bias):
    iden = np.eye(P, dtype=mybir.dt.np(bf16))
    bnw = np.ascontiguousarray(np.asarray(bn_weight, dtype=np.float32))
    bnb = np.ascontiguousarray(np.asarray(bn_bias, dtype=np.float32))
    return [
        {
            "x": np.ascontiguousarray(np.asarray(x[b], dtype=np.float32)),
            "bn_weight": bnw,
            "bn_bias": bnb,
            "iden": iden,
        }
        for b in range(B)
    ]


def kernel(x, prelu1_w, prelu2_w, bn_weight, bn_bias):
    # prelu2 is the identity on the (non-negative) normalized adjacency.
    w1 = float(np.asarray(prelu1_w).reshape(-1)[0])
    nc = _get_nc(w1)
    res = run_bass_kernel_spmd(nc, _in_maps(x, bn_weight, bn_bias), list(range(B)))
    return np.stack([res.results[b]["y"] for b in range(B)]).astype(np.float32)


def kernel_traced(x, prelu1_w, prelu2_w, bn_weight, bn_bias, **trace_kwargs):
    """Same as kernel() but requesting NTFF tracing (if the env supports it)."""
    w1 = float(np.asarray(prelu1_w).reshape(-1)[0])
    nc = _get_nc(w1)
    res = run_bass_kernel_spmd(
        nc, _in_maps(x, bn_weight, bn_bias), list(range(B)), trace=True, **trace_kwargs
    )
    out = np.stack([res.results[b]["y"] for b in range(B)]).astype(np.float32)
    return out, res


# revision 17
# speedup vs baseline: 16.7174x; 16.7174x over previous
"""DGCN dynamic-adjacency kernel for TRN2, data-parallel over batch B=8.

Per core (batch element b):
  h[f]    = mean_L prelu(x[b,f,:])          (phase A: DMA-stream + fused DVE reduce)
  stats   = AllGather h across 8 cores      (phase B: BN batch stats, two-pass var)
  hhat    = (h - mu) * rsqrt(var+eps) * bnw + bnb
  mask    = 1[hhat_n * hhat_m > 0.81] (+I)  (phase C: fused DVE/ACT ops on a
            degree d = row-sum(mask)         partition-broadcast row of hhat)
  c       = rsqrt(d + 1)
  y[n,m]  = mask * c_n * c_m                (phase E: one fused DVE op per chunk)
The output PReLU is the identity since y >= 0 everywhere.  No TensorE use at
all: K=1 outer-product matmuls are ~1us per [128,512] on TRN2, far slower than
streaming the same elements through DVE/ACT with per-partition scalars.
"""

import numpy as np

import concourse.bacc as bacc
import concourse.mybir as mybir
import concourse.tile as tile
from concourse.bass_utils import run_bass_kernel_spmd

B, F, L, P = 8, 2048, 1024, 128
NJ = F // P  # 16 row chunks
THRESH = 0.81
BN_EPS = 1e-5
f32 = mybir.dt.float32
bf16 = mybir.dt.bfloat16

# chunks whose mask is computed on the scalar engine (sign+relu, 2 passes)
# vs the vector engine (fused mult+is_gt, 1 pass); ACT pass is ~1.25x DVE rate.
ACT_CHUNKS = {2, 5, 8, 11, 13, 15}

_NC_CACHE: dict = {}


def _build(w1: float, num_devices: int = B):
    nc = bacc.Bacc(
        "TRN2", target_bir_lowering=False, debug=False, num_devices=num_devices
    )
    x = nc.declare_dram_parameter("x", [F, L], f32, isOutput=False)
    bnw = nc.declare_dram_parameter("bn_weight", [F], f32, isOutput=False)
    bnb = nc.declare_dram_parameter("bn_bias", [F], f32, isOutput=False)
    iden = nc.declare_dram_parameter("iden", [P, P], bf16, isOutput=False)
    y = nc.declare_dram_parameter("y", [F, F], f32, isOutput=True)

    AX = mybir.AxisListType.X
    OP = mybir.AluOpType
    ACT = mybir.ActivationFunctionType
    # prelu(v, w) = max(w*v, v) for w <= 1, min otherwise
    prelu_op = OP.max if w1 <= 1.0 else OP.min

    with tile.TileContext(nc) as tc:
        with (
            tc.tile_pool(name="dram", bufs=1, space="DRAM") as dpool,
            tc.tile_pool(name="xin", bufs=3) as xpool,
            tc.tile_pool(name="small", bufs=1) as spool,
            tc.tile_pool(name="acc", bufs=4) as apool,
            tc.tile_pool(name="mask", bufs=1) as mpool,
            tc.tile_pool(name="sgn", bufs=2) as gpool,
            tc.tile_pool(name="yout", bufs=3) as ypool,
        ):
            # threshold tile for the fused compare (in1 of scalar_tensor_tensor)
            thr = spool.tile([P, F], f32, tag="thr")
            nc.gpsimd.memset(thr[:], THRESH)
            idt = spool.tile([P, P], bf16, tag="idt")
            nc.sync.dma_start(idt[:], iden[:])
            nthr = spool.tile([P, 1], f32, tag="nthr")
            nc.vector.memset(nthr[:], -THRESH)
            epsb = spool.tile([P, 1], f32, tag="epsb")
            nc.vector.memset(epsb[:], BN_EPS)
            bnw_t = spool.tile([P, NJ], f32, tag="bnw_t")
            nc.sync.dma_start(bnw_t[:], bnw[:].rearrange("(j p) -> p j", p=P))
            bnb_t = spool.tile([P, NJ], f32, tag="bnb_t")
            nc.sync.dma_start(bnb_t[:], bnb[:].rearrange("(j p) -> p j", p=P))

            # ---------- phase A: h[f] = mean_L prelu(x[f, :]) ----------
            hsb = spool.tile([P, NJ], f32, tag="hsb")
            for j in range(NJ):
                xt = xpool.tile([P, L], f32, tag="x")
                nc.sync.dma_start(xt[:], x[j * P : (j + 1) * P, :])
                sp = apool.tile([P, 1], f32, tag="sp")
                # xt = prelu(xt) in place, sp = row-sum(prelu(xt))
                nc.vector.scalar_tensor_tensor(
                    xt[:], xt[:], w1, xt[:], op0=OP.mult, op1=prelu_op, accum_out=sp[:]
                )
                nc.vector.tensor_scalar(
                    hsb[:, j : j + 1], sp[:], 1.0 / L, None, op0=OP.mult
                )

            # ---------- phase B: BN batch stats via AllGather ----------
            hd = dpool.tile([F], f32, tag="hd")
            Hd = dpool.tile([B, F], f32, tag="Hd")
            nc.sync.dma_start(hd[:].rearrange("(j p) -> p j", p=P), hsb[:])
            if num_devices > 1:
                nc.gpsimd.collective_compute(
                    "AllGather",
                    OP.bypass,
                    replica_groups=[list(range(B))],
                    ins=[hd[:].opt()],
                    outs=[Hd[:].opt()],
                )
            else:  # single-core timing model variant: fake the gather locally
                for b in range(B):
                    nc.sync.dma_start(
                        Hd[b : b + 1, :], hd[:].rearrange("(o f) -> o f", o=1)
                    )
            # Hp[p, b*NJ + j] = Hd[b, j*128+p] — one balanced 3-dim DMA
            Hp = spool.tile([P, B * NJ], f32, tag="Hp")
            nc.sync.dma_start(
                Hp[:].rearrange("p (b j) -> p b j", j=NJ),
                Hd[:].rearrange("b (j p) -> p b j", p=P),
            )
            Hp3 = Hp[:].rearrange("p (b j) -> p j b", j=NJ)
            smu = spool.tile([P, NJ], f32, tag="smu")
            nc.vector.tensor_reduce(smu[:], Hp3, axis=AX, op=OP.add)
            mu = spool.tile([P, NJ], f32, tag="mu")
            nc.vector.tensor_scalar(mu[:], smu[:], 1.0 / B, None, op0=OP.mult)
            # two-pass variance: hcall = Hp - mu (mu free-dim-broadcast over b)
            mu3b = mu[:].rearrange("p (o j) -> p o j", o=1).to_broadcast([P, B, NJ])
            hcall = spool.tile([P, B * NJ], f32, tag="hcall")
            nc.vector.tensor_sub(
                hcall[:].rearrange("p (b j) -> p b j", j=NJ),
                Hp[:].rearrange("p (b j) -> p b j", j=NJ),
                mu3b,
            )
            hsq = spool.tile([P, B * NJ], f32, tag="hsq")
            nc.vector.tensor_mul(hsq[:], hcall[:], hcall[:])
            ssq = spool.tile([P, NJ], f32, tag="ssq")
            nc.vector.tensor_reduce(
                ssq[:], hsq[:].rearrange("p (b j) -> p j b", j=NJ), axis=AX, op=OP.add
            )
            # std = sqrt(ssq/B + eps): fold the 1/B into the activation scale
            stdt = spool.tile([P, NJ], f32, tag="stdt")
            nc.scalar.activation(
                stdt[:], ssq[:], ACT.Sqrt, bias=epsb[:], scale=1.0 / B
            )
            inv = spool.tile([P, NJ], f32, tag="inv")
            nc.vector.reciprocal(inv[:], stdt[:])
            # hh2 = ((h - mu) * inv) * bnw + bnb, matching the reference's order
            hc = spool.tile([P, NJ], f32, tag="hc")
            nc.vector.tensor_sub(hc[:], hsb[:], mu[:])
            hn = spool.tile([P, NJ], f32, tag="hn")
            nc.vector.tensor_mul(hn[:], hc[:], inv[:])
            hw = spool.tile([P, NJ], f32, tag="hw")
            nc.vector.tensor_mul(hw[:], hn[:], bnw_t[:])
            hh2 = spool.tile([P, NJ], f32, tag="hh2")
            nc.vector.tensor_add(hh2[:], hw[:], bnb_t[:])
            hhd = dpool.tile([F], f32, tag="hhd")
            nc.sync.dma_start(hhd[:].rearrange("(j p) -> p j", p=P), hh2[:])
            # hb[p, m] = hhat[m] for every partition p (broadcast row)
            hb = spool.tile([P, F], f32, tag="hb")
            nc.sync.dma_start(
                hb[:], hhd[:].rearrange("(o f) -> o f", o=1).to_broadcast([P, F])
            )

            # ---------- phase C: mask + degrees ----------
            dsb = spool.tile([P, NJ], f32, tag="dsb")
            masks = []
            for i in range(NJ):
                mt = mpool.tile([P, F], bf16, tag=f"m{i}", name=f"mask{i}")
                if i in ACT_CHUNKS:
                    # sgn = sign(hb*h_n - thresh) in {-1,0,1}; mask = relu(sgn)
                    sg = gpool.tile([P, F], bf16, tag="sg", name=f"sg{i}")
                    nc.scalar.activation(
                        sg[:], hb[:], ACT.Sign, bias=nthr[:], scale=hh2[:, i : i + 1]
                    )
                    nc.scalar.activation(
                        mt[:], sg[:], ACT.Relu, accum_out=dsb[:, i : i + 1]
                    )
                else:
                    # mask = (hb * h_n) > thr, degree = row-sum(mask)
                    nc.vector.scalar_tensor_tensor(
                        mt[:],
                        hb[:],
                        hh2[:, i : i + 1],
                        thr[:],
                        op0=OP.mult,
                        op1=OP.is_gt,
                        accum_out=dsb[:, i : i + 1],
                    )
                # add identity on the diagonal block
                nc.vector.tensor_add(
                    mt[:, i * P : (i + 1) * P], mt[:, i * P : (i + 1) * P], idt[:]
                )
                masks.append(mt)

            # ---------- phase D: c = rsqrt(d + 1) ----------
            sqd = spool.tile([P, NJ], f32, tag="sqd")
            nc.scalar.activation(sqd[:], dsb[:], ACT.Sqrt, bias=1.0)
            csb = spool.tile([P, NJ], f32, tag="csb")
            nc.vector.reciprocal(csb[:], sqd[:])
            cd = dpool.tile([F], f32, tag="cd")
            nc.sync.dma_start(cd[:].rearrange("(j p) -> p j", p=P), csb[:])
            cbt = spool.tile([P, F], f32, tag="cbt")
            nc.sync.dma_start(
                cbt[:], cd[:].rearrange("(o f) -> o f", o=1).to_broadcast([P, F])
            )

            # ---------- phase E: y = mask * c_n * c_m ----------
            for i in range(NJ):
                yt = ypool.tile([P, F], f32, tag="yt")
                nc.vector.scalar_tensor_tensor(
                    yt[:],
                    cbt[:],
                    csb[:, i : i + 1],
                    masks[i][:],
                    op0=OP.mult,
                    op1=OP.mult,
                )
                nc.sync.dma_start(y[i * P : (i + 1) * P, :], yt[:])

    nc.compile()
    return nc


def _get_nc(w1: float):
    key = round(w1, 9)
    if key not in _NC_CACHE:
        _NC_CACHE[key] = _build(w1)
    return _NC_CACHE[key]


def _in_maps(x, bn_weight, bn_bias):
    iden = np.eye(P, dtype=mybir.dt.np(bf16))
    bnw = np.ascontiguousarray(np.asarray(bn_weight, dtype=np.float32))
    bnb = np.ascontiguousarray(np.asarray(bn_bias, dtype=np.float32))
    return [
        {
            "x": np.ascontiguousarray(np.asarray(x[b], dtype=np.float32)),
            "bn_weight": bnw,
            "bn_bias": bnb,
            "iden": iden,
        }
        for b in range(B)
    ]


def kernel(x, prelu1_w, prelu2_w, bn_weight, bn_bias):
    # prelu2 is the identity on the (non-negative) normalized adjacency.
    w1 = float(np.asarray(prelu1_w).reshape(-1)[0])
    nc = _get_nc(w1)
    res = run_bass_kernel_spmd(nc, _in_maps(x, bn_weight, bn_bias), list(range(B)))
    return np.stack([res.results[b]["y"] for b in range(B)]).astype(np.float32)


def kernel_traced(x, prelu1_w, prelu2_w, bn_weight, bn_bias, **trace_kwargs):
    """Same as kernel() but requesting NTFF tracing (if the env supports it)."""
    w1 = float(np.asarray(prelu1_w).reshape(-1)[0])
    nc = _get_nc(w1)
    res = run_bass_kernel_spmd(
        nc, _in_maps(x, bn_weight, bn_bias), list(range(B)), trace=True, **trace_kwargs
    )
    out = np.stack([res.results[b]["y"] for b in range(B)]).astype(np.float32)
    return out, res


# revision 25
# speedup vs baseline: 19.9188x; 1.1915x over previous
"""DGCN dynamic-adjacency kernel for TRN2, data-parallel over batch B=8.

Per core (batch element b):
  h[f]    = mean_L prelu(x[b,f,:])          (phase A: DMA-stream + fused DVE reduce)
  stats   = AllGather h across 8 cores      (phase B: BN batch stats, two-pass var)
  hhat    = (h - mu) * rsqrt(var+eps) * bnw + bnb
  mask    = 1[hhat_n * hhat_m > 0.81] (+I)  (phase C: fused DVE/ACT ops on a
            degree d = row-sum(mask)         partition-broadcast row of hhat)
  c       = rsqrt(d + 1)
  y[n,m]  = mask * c_n * c_m                (phase E: one fused DVE op per chunk)
The output PReLU is the identity since y >= 0 everywhere.  No TensorE use at
all: K=1 outer-product matmuls are ~1us per [128,512] on TRN2, far slower than
streaming the same elements through DVE/ACT with per-partition scalars.
"""

import numpy as np

import concourse.bacc as bacc
import concourse.mybir as mybir
import concourse.tile as tile
from concourse.bass_utils import run_bass_kernel_spmd

B, F, L, P = 8, 2048, 1024, 128
NJ = F // P  # 16 row chunks
THRESH = 0.81
BN_EPS = 1e-5
f32 = mybir.dt.float32
bf16 = mybir.dt.bfloat16

# chunks whose mask is computed on the scalar engine vs the vector engine.
# Both are single-pass: DVE does fused mult+is_gt+accum; ACT computes
# sigmoid(K_SAT*(h_m*h_n - thresh) - SAT_MARGIN), which saturates to exact
# 0/1 beyond ~3ulp of the threshold (and ~0 for an exactly-equal product,
# matching the strict '>').  ACT is ~1.15x the DVE per-pass rate, so it
# carries 9 of the 16 chunks.
ACT_CHUNKS = {1, 3, 5, 7, 9, 11, 13, 14, 15}
K_SAT = 1.0e10
SAT_MARGIN = 37.0

_NC_CACHE: dict = {}


def _build(w1: float, num_devices: int = B):
    nc = bacc.Bacc(
        "TRN2", target_bir_lowering=False, debug=False, num_devices=num_devices
    )
    x = nc.declare_dram_parameter("x", [F, L], f32, isOutput=False)
    bnw = nc.declare_dram_parameter("bn_weight", [F], f32, isOutput=False)
    bnb = nc.declare_dram_parameter("bn_bias", [F], f32, isOutput=False)
    iden = nc.declare_dram_parameter("iden", [P, P], bf16, isOutput=False)
    y = nc.declare_dram_parameter("y", [F, F], f32, isOutput=True)

    AX = mybir.AxisListType.X
    OP = mybir.AluOpType
    ACT = mybir.ActivationFunctionType
    # prelu(v, w) = max(w*v, v) for w <= 1, min otherwise
    prelu_op = OP.max if w1 <= 1.0 else OP.min

    with tile.TileContext(nc) as tc:
        with (
            tc.tile_pool(name="dram", bufs=1, space="DRAM") as dpool,
            tc.tile_pool(name="xin", bufs=3) as xpool,
            tc.tile_pool(name="small", bufs=1) as spool,
            tc.tile_pool(name="acc", bufs=4) as apool,
            tc.tile_pool(name="mask", bufs=1) as mpool,
            tc.tile_pool(name="yout", bufs=4) as ypool,
        ):
            # threshold tile for the fused compare (in1 of scalar_tensor_tensor)
            thr = spool.tile([P, F], f32, tag="thr")
            nc.gpsimd.memset(thr[:], THRESH)
            idt = spool.tile([P, P], bf16, tag="idt")
            nc.sync.dma_start(idt[:], iden[:])
            bsig = spool.tile([P, 1], f32, tag="bsig")
            nc.vector.memset(bsig[:], -THRESH * K_SAT - SAT_MARGIN)
            epsb = spool.tile([P, 1], f32, tag="epsb")
            nc.vector.memset(epsb[:], BN_EPS)
            bnw_t = spool.tile([P, NJ], f32, tag="bnw_t")
            nc.sync.dma_start(bnw_t[:], bnw[:].rearrange("(j p) -> p j", p=P))
            bnb_t = spool.tile([P, NJ], f32, tag="bnb_t")
            nc.sync.dma_start(bnb_t[:], bnb[:].rearrange("(j p) -> p j", p=P))

            # ---------- phase A: h[f] = mean_L prelu(x[f, :]) ----------
            # 1 MiB slab loads: chunks (2s, 2s+1) land side by side in one
            # [128, 2L] tile (fewer DMA issues); the fused prelu+row-sum then
            # runs once per chunk on the column halves.
            hsb = spool.tile([P, NJ], f32, tag="hsb")
            xv = x[:].rearrange("(s h p) l -> s p h l", p=P, h=2)
            for s in range(NJ // 2):
                xt = xpool.tile([P, 2 * L], f32, tag="x")
                dma = nc.sync.dma_start if s % 2 == 0 else nc.gpsimd.dma_start
                dma(xt[:].rearrange("p (h l) -> p h l", h=2), xv[s])
                for hh in range(2):
                    j = 2 * s + hh
                    sp = apool.tile([P, 1], f32, tag="sp")
                    xs = xt[:, hh * L : (hh + 1) * L]
                    # xs = prelu(xs) in place, sp = row-sum(prelu(xs))
                    nc.vector.scalar_tensor_tensor(
                        xs, xs, w1, xs, op0=OP.mult, op1=prelu_op, accum_out=sp[:]
                    )
                    nc.vector.tensor_scalar(
                        hsb[:, j : j + 1], sp[:], 1.0 / L, None, op0=OP.mult
                    )

            # ---------- phase B: BN batch stats via AllGather ----------
            # hd holds h in p-major order (hd[p*NJ + j] = h[j*128 + p]) so the
            # post-gather Hp load reads contiguous 64B runs instead of 4B
            # elements; only this vector's internal order changes.
            hd = dpool.tile([F], f32, tag="hd")
            Hd = dpool.tile([B, F], f32, tag="Hd")
            nc.sync.dma_start(hd[:].rearrange("(p j) -> p j", j=NJ), hsb[:])
            if num_devices > 1:
                nc.gpsimd.collective_compute(
                    "AllGather",
                    OP.bypass,
                    replica_groups=[list(range(B))],
                    ins=[hd[:].opt()],
                    outs=[Hd[:].opt()],
                )
            else:  # single-core timing model variant: fake the gather locally
                for b in range(B):
                    nc.sync.dma_start(
                        Hd[b : b + 1, :], hd[:].rearrange("(o f) -> o f", o=1)
                    )
            # Hp[p, b*NJ + j] = Hd[b, p*NJ+j] — one balanced 3-dim DMA with
            # 64B contiguous runs (j innermost on both sides)
            Hp = spool.tile([P, B * NJ], f32, tag="Hp")
            nc.sync.dma_start(
                Hp[:].rearrange("p (b j) -> p b j", j=NJ),
                Hd[:].rearrange("b (p j) -> p b j", j=NJ),
            )
            Hp3 = Hp[:].rearrange("p (b j) -> p j b", j=NJ)
            smu = spool.tile([P, NJ], f32, tag="smu")
            nc.vector.tensor_reduce(smu[:], Hp3, axis=AX, op=OP.add)
            mu = spool.tile([P, NJ], f32, tag="mu")
            nc.vector.tensor_scalar(mu[:], smu[:], 1.0 / B, None, op0=OP.mult)
            # two-pass variance: hcall = Hp - mu (mu free-dim-broadcast over b)
            mu3b = mu[:].rearrange("p (o j) -> p o j", o=1).to_broadcast([P, B, NJ])
            hcall = spool.tile([P, B * NJ], f32, tag="hcall")
            nc.vector.tensor_sub(
                hcall[:].rearrange("p (b j) -> p b j", j=NJ),
                Hp[:].rearrange("p (b j) -> p b j", j=NJ),
                mu3b,
            )
            hsq = spool.tile([P, B * NJ], f32, tag="hsq")
            nc.vector.tensor_mul(hsq[:], hcall[:], hcall[:])
            ssq = spool.tile([P, NJ], f32, tag="ssq")
            nc.vector.tensor_reduce(
                ssq[:], hsq[:].rearrange("p (b j) -> p j b", j=NJ), axis=AX, op=OP.add
            )
            # std = sqrt(ssq/B + eps): fold the 1/B into the activation scale
            stdt = spool.tile([P, NJ], f32, tag="stdt")
            nc.scalar.activation(
                stdt[:], ssq[:], ACT.Sqrt, bias=epsb[:], scale=1.0 / B
            )
            inv = spool.tile([P, NJ], f32, tag="inv")
            nc.vector.reciprocal(inv[:], stdt[:])
            # hh2 = ((h - mu) * inv) * bnw + bnb, matching the reference's order
            hc = spool.tile([P, NJ], f32, tag="hc")
            nc.vector.tensor_sub(hc[:], hsb[:], mu[:])
            hn = spool.tile([P, NJ], f32, tag="hn")
            nc.vector.tensor_mul(hn[:], hc[:], inv[:])
            hw = spool.tile([P, NJ], f32, tag="hw")
            nc.vector.tensor_mul(hw[:], hn[:], bnw_t[:])
            hh2 = spool.tile([P, NJ], f32, tag="hh2")
            nc.vector.tensor_add(hh2[:], hw[:], bnb_t[:])
            hhd = dpool.tile([F], f32, tag="hhd")
            nc.sync.dma_start(hhd[:].rearrange("(j p) -> p j", p=P), hh2[:])
            # hb[p, m] = hhat[m] for every partition p (broadcast row)
            hb = spool.tile([P, F], f32, tag="hb")
            nc.sync.dma_start(
                hb[:], hhd[:].rearrange("(o f) -> o f", o=1).to_broadcast([P, F])
            )

            # ---------- phase C: mask + degrees ----------
            # khh = K_SAT * hhat for the saturated-sigmoid scale
            khh = spool.tile([P, NJ], f32, tag="khh")
            nc.vector.tensor_scalar(khh[:], hh2[:], K_SAT, None, op0=OP.mult)
            dsb = spool.tile([P, NJ], f32, tag="dsb")
            masks = []
            for i in range(NJ):
                mt = mpool.tile([P, F], bf16, tag=f"m{i}", name=f"mask{i}")
                if i in ACT_CHUNKS:
                    # mask = sigmoid(K*(h_m*h_n - thresh) - margin) in {0,1}
                    nc.scalar.activation(
                        mt[:],
                        hb[:],
                        ACT.Sigmoid,
                        bias=bsig[:],
                        scale=khh[:, i : i + 1],
                        accum_out=dsb[:, i : i + 1],
                    )
                else:
                    # mask = (hb * h_n) > thr, degree = row-sum(mask)
                    nc.vector.scalar_tensor_tensor(
                        mt[:],
                        hb[:],
                        hh2[:, i : i + 1],
                        thr[:],
                        op0=OP.mult,
                        op1=OP.is_gt,
                        accum_out=dsb[:, i : i + 1],
                    )
                # add identity on the diagonal block
                nc.vector.tensor_add(
                    mt[:, i * P : (i + 1) * P], mt[:, i * P : (i + 1) * P], idt[:]
                )
                masks.append(mt)

            # ---------- phase D: c = rsqrt(d + 1) ----------
            sqd = spool.tile([P, NJ], f32, tag="sqd")
            nc.scalar.activation(sqd[:], dsb[:], ACT.Sqrt, bias=1.0)
            csb = spool.tile([P, NJ], f32, tag="csb")
            nc.vector.reciprocal(csb[:], sqd[:])
            cd = dpool.tile([F], f32, tag="cd")
            nc.sync.dma_start(cd[:].rearrange("(j p) -> p j", p=P), csb[:])
            cbt = spool.tile([P, F], f32, tag="cbt")
            nc.sync.dma_start(
                cbt[:], cd[:].rearrange("(o f) -> o f", o=1).to_broadcast([P, F])
            )

            # ---------- phase E: y = mask * c_n * c_m ----------
            for i in range(NJ):
                yt = ypool.tile([P, F], f32, tag="yt")
                nc.vector.scalar_tensor_tensor(
                    yt[:],
                    cbt[:],
                    csb[:, i : i + 1],
                    masks[i][:],
                    op0=OP.mult,
                    op1=OP.mult,
                )
                # alternate DMA queues so descriptor issue overlaps transfer
                dma = nc.sync.dma_start if i % 2 == 0 else nc.gpsimd.dma_start
                dma(y[i * P : (i + 1) * P, :], yt[:])

    nc.compile()
    return nc


def _get_nc(w1: float):
    key = round(w1, 9)
    if key not in _NC_CACHE:
        _NC_CACHE[key] = _build(w1)
    return _NC_CACHE[key]


def _in_maps(x, bn_weight, bn_bias):
    iden = np.eye(P, dtype=mybir.dt.np(bf16))
    bnw = np.ascontiguousarray(np.asarray(bn_weight, dtype=np.float32))
    bnb = np.ascontiguousarray(np.asarray(bn_bias, dtype=np.float32))
    return [
        {
            "x": np.ascontiguousarray(np.asarray(x[b], dtype=np.float32)),
            "bn_weight": bnw,
            "bn_bias": bnb,
            "iden": iden,
        }
        for b in range(B)
    ]


def kernel(x, prelu1_w, prelu2_w, bn_weight, bn_bias):
    # prelu2 is the identity on the (non-negative) normalized adjacency.
    w1 = float(np.asarray(prelu1_w).reshape(-1)[0])
    nc = _get_nc(w1)
    res = run_bass_kernel_spmd(nc, _in_maps(x, bn_weight, bn_bias), list(range(B)))
    return np.stack([res.results[b]["y"] for b in range(B)]).astype(np.float32)


def kernel_traced(x, prelu1_w, prelu2_w, bn_weight, bn_bias, **trace_kwargs):
    """Same as kernel() but requesting NTFF tracing (if the env supports it)."""
    w1 = float(np.asarray(prelu1_w).reshape(-1)[0])
    nc = _get_nc(w1)
    res = run_bass_kernel_spmd(
        nc, _in_maps(x, bn_weight, bn_bias), list(range(B)), trace=True, **trace_kwargs
    )
    out = np.stack([res.results[b]["y"] for b in range(B)]).astype(np.float32)
    return out, res


# revision 29
# speedup vs baseline: 30.7814x; 1.5453x over previous
"""DGCN dynamic-adjacency kernel for TRN2, data-parallel over batch B=8.

Per core (batch element b):
  h[f]    = mean_L prelu(x[b,f,:])          (phase A: DMA-stream + fused DVE reduce)
  stats   = AllGather h across 8 cores      (phase B: BN batch stats, two-pass var)
  hhat    = (h - mu) * rsqrt(var+eps) * bnw + bnb
  mask    = 1[hhat_n * hhat_m > 0.81] (+I)  (phase C: fused DVE/ACT ops on a
            degree d = row-sum(mask)         partition-broadcast row of hhat)
  c       = rsqrt(d + 1)
  y[n,m]  = mask * c_n * c_m                (phase E: one fused DVE op per chunk)
The output PReLU is the identity since y >= 0 everywhere.  No TensorE use at
all: K=1 outer-product matmuls are ~1us per [128,512] on TRN2, far slower than
streaming the same elements through DVE/ACT with per-partition scalars.
"""

import numpy as np

import concourse.bacc as bacc
import concourse.mybir as mybir
import concourse.tile as tile
from concourse.bass_utils import run_bass_kernel_spmd

B, F, L, P = 8, 2048, 1024, 128
NJ = F // P  # 16 row chunks
THRESH = 0.81
BN_EPS = 1e-5
f32 = mybir.dt.float32
bf16 = mybir.dt.bfloat16

# chunks whose mask is computed on the scalar engine vs the vector engine.
# Both are single-pass: DVE does fused mult+is_gt+accum; ACT computes
# sigmoid(K_SAT*(h_m*h_n - thresh) - SAT_MARGIN), which saturates to exact
# 0/1 beyond ~3ulp of the threshold (and ~0 for an exactly-equal product,
# matching the strict '>').  ACT is ~1.15x the DVE per-pass rate, so it
# carries 9 of the 16 chunks.
ACT_CHUNKS = {0, 2, 4, 6, 8, 10, 12, 14}
K_SAT = 1.0e10
SAT_MARGIN = 37.0

_NC_CACHE: dict = {}


def _build(w1: float, num_devices: int = B):
    nc = bacc.Bacc(
        "TRN2", target_bir_lowering=False, debug=False, num_devices=num_devices
    )
    x = nc.declare_dram_parameter("x", [F, L], f32, isOutput=False)
    bnw = nc.declare_dram_parameter("bn_weight", [F], f32, isOutput=False)
    bnb = nc.declare_dram_parameter("bn_bias", [F], f32, isOutput=False)
    iden = nc.declare_dram_parameter("iden", [P, P], bf16, isOutput=False)
    y = nc.declare_dram_parameter("y", [F, F], f32, isOutput=True)

    AX = mybir.AxisListType.X
    OP = mybir.AluOpType
    ACT = mybir.ActivationFunctionType
    # prelu(v, w) = max(w*v, v) for w <= 1, min otherwise
    prelu_op = OP.max if w1 <= 1.0 else OP.min

    with tile.TileContext(nc) as tc:
        with (
            tc.tile_pool(name="dram", bufs=1, space="DRAM") as dpool,
            tc.tile_pool(name="xin", bufs=3) as xpool,
            tc.tile_pool(name="small", bufs=1) as spool,
            tc.tile_pool(name="acc", bufs=4) as apool,
            tc.tile_pool(name="mask", bufs=1) as mpool,
            tc.tile_pool(name="yout", bufs=4) as ypool,
        ):
            # threshold tile for the fused compare (in1 of scalar_tensor_tensor)
            thr = spool.tile([P, F], f32, tag="thr")
            nc.gpsimd.memset(thr[:], THRESH)
            idt = spool.tile([P, P], bf16, tag="idt")
            nc.gpsimd.dma_start(idt[:], iden[:])
            bsig = spool.tile([P, 1], f32, tag="bsig")
            nc.vector.memset(bsig[:], -THRESH * K_SAT - SAT_MARGIN)
            epsb = spool.tile([P, 1], f32, tag="epsb")
            nc.vector.memset(epsb[:], BN_EPS)
            bnw_t = spool.tile([P, NJ], f32, tag="bnw_t")
            nc.gpsimd.dma_start(bnw_t[:], bnw[:].rearrange("(j p) -> p j", p=P))
            bnb_t = spool.tile([P, NJ], f32, tag="bnb_t")
            nc.gpsimd.dma_start(bnb_t[:], bnb[:].rearrange("(j p) -> p j", p=P))

            # ---------- phase A: h[f] = mean_L prelu(x[f, :]) ----------
            # 1 MiB slab loads: chunks (2s, 2s+1) land side by side in one
            # [128, 2L] tile (fewer DMA issues); the fused prelu+row-sum then
            # runs once per chunk on the column halves.
            hsb = spool.tile([P, NJ], f32, tag="hsb")
            xv = x[:].rearrange("(s h p) l -> s p h l", p=P, h=2)
            for s in range(NJ // 2):
                xt = xpool.tile([P, 2 * L], f32, tag="x")
                dma = nc.sync.dma_start if s % 2 == 0 else nc.gpsimd.dma_start
                dma(xt[:].rearrange("p (h l) -> p h l", h=2), xv[s])
                for hh in range(2):
                    j = 2 * s + hh
                    sp = apool.tile([P, 1], f32, tag="sp")
                    xs = xt[:, hh * L : (hh + 1) * L]
                    # xs = prelu(xs) in place, sp = row-sum(prelu(xs))
                    nc.vector.scalar_tensor_tensor(
                        xs, xs, w1, xs, op0=OP.mult, op1=prelu_op, accum_out=sp[:]
                    )
                    nc.vector.tensor_scalar(
                        hsb[:, j : j + 1], sp[:], 1.0 / L, None, op0=OP.mult
                    )

            # ---------- phase B: BN batch stats via AllGather ----------
            # hd holds h in p-major order (hd[p*NJ + j] = h[j*128 + p]) so the
            # post-gather Hp load reads contiguous 64B runs instead of 4B
            # elements; only this vector's internal order changes.
            hd = dpool.tile([F], f32, tag="hd")
            Hd = dpool.tile([B, F], f32, tag="Hd")
            nc.sync.dma_start(hd[:].rearrange("(p j) -> p j", j=NJ), hsb[:])
            if num_devices > 1:
                nc.gpsimd.collective_compute(
                    "AllGather",
                    OP.bypass,
                    replica_groups=[list(range(B))],
                    ins=[hd[:].opt()],
                    outs=[Hd[:].opt()],
                )
            else:  # single-core timing model variant: fake the gather locally
                for b in range(B):
                    nc.sync.dma_start(
                        Hd[b : b + 1, :], hd[:].rearrange("(o f) -> o f", o=1)
                    )
            # Hp[p, b*NJ + j] = Hd[b, p*NJ+j] — one balanced 3-dim DMA with
            # 64B contiguous runs (j innermost on both sides)
            Hp = spool.tile([P, B * NJ], f32, tag="Hp")
            nc.sync.dma_start(
                Hp[:].rearrange("p (b j) -> p b j", j=NJ),
                Hd[:].rearrange("b (p j) -> p b j", j=NJ),
            )
            Hp3 = Hp[:].rearrange("p (b j) -> p j b", j=NJ)
            smu = spool.tile([P, NJ], f32, tag="smu")
            nc.vector.tensor_reduce(smu[:], Hp3, axis=AX, op=OP.add)
            mu = spool.tile([P, NJ], f32, tag="mu")
            nc.vector.tensor_scalar(mu[:], smu[:], 1.0 / B, None, op0=OP.mult)
            # two-pass variance: hcall = Hp - mu (mu free-dim-broadcast over b)
            mu3b = mu[:].rearrange("p (o j) -> p o j", o=1).to_broadcast([P, B, NJ])
            hcall = spool.tile([P, B * NJ], f32, tag="hcall")
            nc.vector.tensor_sub(
                hcall[:].rearrange("p (b j) -> p b j", j=NJ),
                Hp[:].rearrange("p (b j) -> p b j", j=NJ),
                mu3b,
            )
            hsq = spool.tile([P, B * NJ], f32, tag="hsq")
            nc.vector.tensor_mul(hsq[:], hcall[:], hcall[:])
            ssq = spool.tile([P, NJ], f32, tag="ssq")
            nc.vector.tensor_reduce(
                ssq[:], hsq[:].rearrange("p (b j) -> p j b", j=NJ), axis=AX, op=OP.add
            )
            # std = sqrt(ssq/B + eps): fold the 1/B into the activation scale
            stdt = spool.tile([P, NJ], f32, tag="stdt")
            nc.scalar.activation(
                stdt[:], ssq[:], ACT.Sqrt, bias=epsb[:], scale=1.0 / B
            )
            inv = spool.tile([P, NJ], f32, tag="inv")
            nc.vector.reciprocal(inv[:], stdt[:])
            # hh2 = ((h - mu) * inv) * bnw + bnb, matching the reference's order
            hc = spool.tile([P, NJ], f32, tag="hc")
            nc.vector.tensor_sub(hc[:], hsb[:], mu[:])
            hn = spool.tile([P, NJ], f32, tag="hn")
            nc.vector.tensor_mul(hn[:], hc[:], inv[:])
            hw = spool.tile([P, NJ], f32, tag="hw")
            nc.vector.tensor_mul(hw[:], hn[:], bnw_t[:])
            hh2 = spool.tile([P, NJ], f32, tag="hh2")
            nc.vector.tensor_add(hh2[:], hw[:], bnb_t[:])
            hhd = dpool.tile([F], f32, tag="hhd")
            nc.sync.dma_start(hhd[:].rearrange("(j p) -> p j", p=P), hh2[:])
            # hb[p, m] = hhat[m] for every partition p (broadcast row)
            hb = spool.tile([P, F], f32, tag="hb")
            nc.sync.dma_start(
                hb[:], hhd[:].rearrange("(o f) -> o f", o=1).to_broadcast([P, F])
            )

            # ---------- phase C: mask + degrees ----------
            # khh = K_SAT * hhat for the saturated-sigmoid scale
            khh = spool.tile([P, NJ], f32, tag="khh")
            nc.vector.tensor_scalar(khh[:], hh2[:], K_SAT, None, op0=OP.mult)
            dsb = spool.tile([P, NJ], f32, tag="dsb")
            masks = []
            for i in range(NJ):
                mt = mpool.tile([P, F], bf16, tag=f"m{i}", name=f"mask{i}")
                if i in ACT_CHUNKS:
                    # mask = sigmoid(K*(h_m*h_n - thresh) - margin) in {0,1}
                    nc.scalar.activation(
                        mt[:],
                        hb[:],
                        ACT.Sigmoid,
                        bias=bsig[:],
                        scale=khh[:, i : i + 1],
                        accum_out=dsb[:, i : i + 1],
                    )
                else:
                    # mask = (hb * h_n) > thr, degree = row-sum(mask)
                    nc.vector.scalar_tensor_tensor(
                        mt[:],
                        hb[:],
                        hh2[:, i : i + 1],
                        thr[:],
                        op0=OP.mult,
                        op1=OP.is_gt,
                        accum_out=dsb[:, i : i + 1],
                    )
                # add identity on the diagonal block (GpSimd: DVE/ACT are the
                # phase-C bottleneck, POOL is idle)
                nc.gpsimd.tensor_add(
                    mt[:, i * P : (i + 1) * P], mt[:, i * P : (i + 1) * P], idt[:]
                )
                masks.append(mt)

            # ---------- phase D: c = rsqrt(d + 1) ----------
            sqd = spool.tile([P, NJ], f32, tag="sqd")
            nc.scalar.activation(sqd[:], dsb[:], ACT.Sqrt, bias=1.0)
            csb = spool.tile([P, NJ], f32, tag="csb")
            nc.vector.reciprocal(csb[:], sqd[:])
            cd = dpool.tile([F], f32, tag="cd")
            nc.sync.dma_start(cd[:].rearrange("(j p) -> p j", p=P), csb[:])
            cbt = spool.tile([P, F], f32, tag="cbt")
            nc.sync.dma_start(
                cbt[:], cd[:].rearrange("(o f) -> o f", o=1).to_broadcast([P, F])
            )

            # ---------- phase E: y = mask * c_n * c_m ----------
            for i in range(NJ):
                yt = ypool.tile([P, F], f32, tag="yt")
                nc.vector.scalar_tensor_tensor(
                    yt[:],
                    cbt[:],
                    csb[:, i : i + 1],
                    masks[i][:],
                    op0=OP.mult,
                    op1=OP.mult,
                )
                # alternate DMA queues so descriptor issue overlaps transfer
                dma = nc.sync.dma_start if i % 2 == 0 else nc.gpsimd.dma_start
                dma(y[i * P : (i + 1) * P, :], yt[:])

    nc.compile()
    return nc


def _get_nc(w1: float):
    key = round(w1, 9)
    if key not in _NC_CACHE:
        _NC_CACHE[key] = _build(w1)
    return _NC_CACHE[key]


def _in_maps(x, bn_weight, bn_bias):
    iden = np.eye(P, dtype=mybir.dt.np(bf16))
    bnw = np.ascontiguousarray(np.asarray(bn_weight, dtype=np.float32))
    bnb = np.ascontiguousarray(np.asarray(bn_bias, dtype=np.float32))
    return [
        {
            "x": np.ascontiguousarray(np.asarray(x[b], dtype=np.float32)),
            "bn_weight": bnw,
            "bn_bias": bnb,
            "iden": iden,
        }
        for b in range(B)
    ]


def kernel(x, prelu1_w, prelu2_w, bn_weight, bn_bias):
    # prelu2 is the identity on the (non-negative) normalized adjacency.
    w1 = float(np.asarray(prelu1_w).reshape(-1)[0])
    nc = _get_nc(w1)
    res = run_bass_kernel_spmd(nc, _in_maps(x, bn_weight, bn_bias), list(range(B)))
    return np.stack([res.results[b]["y"] for b in range(B)]).astype(np.float32)


def kernel_traced(x, prelu1_w, prelu2_w, bn_weight, bn_bias, **trace_kwargs):
    """Same as kernel() but requesting NTFF tracing (if the env supports it)."""
    w1 = float(np.asarray(prelu1_w).reshape(-1)[0])
    nc = _get_nc(w1)
    res = run_bass_kernel_spmd(
        nc, _in_maps(x, bn_weight, bn_bias), list(range(B)), trace=True, **trace_kwargs
    )
    out = np.stack([res.results[b]["y"] for b in range(B)]).astype(np.float32)
    return out, res


# revision 32
# speedup vs baseline: 74.1233x; 2.4081x over previous
"""DGCN dynamic-adjacency kernel for TRN2, data-parallel over batch B=8.

Per core (batch element b):
  h[f]    = mean_L prelu(x[b,f,:])          (phase A: DMA-stream + fused DVE reduce)
  stats   = AllGather h across 8 cores      (phase B: BN batch stats, two-pass var)
  hhat    = (h - mu) * rsqrt(var+eps) * bnw + bnb
  mask    = 1[hhat_n * hhat_m > 0.81] (+I)  (phase C: fused DVE/ACT ops on a
            degree d = row-sum(mask)         partition-broadcast row of hhat)
  c       = rsqrt(d + 1)
  y[n,m]  = mask * c_n * c_m                (phase E: one fused DVE op per chunk)
The output PReLU is the identity since y >= 0 everywhere.  No TensorE use at
all: K=1 outer-product matmuls are ~1us per [128,512] on TRN2, far slower than
streaming the same elements through DVE/ACT with per-partition scalars.
"""

import numpy as np

import concourse.bacc as bacc
import concourse.mybir as mybir
import concourse.tile as tile
from concourse.bass_utils import run_bass_kernel_spmd

B, F, L, P = 8, 2048, 1024, 128
NJ = F // P  # 16 row chunks
THRESH = 0.81
BN_EPS = 1e-5
f32 = mybir.dt.float32
bf16 = mybir.dt.bfloat16

# chunks whose mask is computed on the scalar engine vs the vector engine.
# Both are single-pass: DVE does fused mult+is_gt+accum; ACT computes
# sigmoid(K_SAT*(h_m*h_n - thresh) - SAT_MARGIN), which saturates to exact
# 0/1 beyond ~3ulp of the threshold (and ~0 for an exactly-equal product,
# matching the strict '>').  ACT is ~1.15x the DVE per-pass rate, so it
# carries 9 of the 16 chunks.
ACT_CHUNKS = {0, 2, 4, 6, 8, 10, 12, 14}
K_SAT = 1.0e10
SAT_MARGIN = 37.0

_NC_CACHE: dict = {}


def _build(w1: float, num_devices: int = B):
    nc = bacc.Bacc(
        "TRN2", target_bir_lowering=False, debug=False, num_devices=num_devices
    )
    x = nc.declare_dram_parameter("x", [F, L], f32, isOutput=False)
    bnw = nc.declare_dram_parameter("bn_weight", [F], f32, isOutput=False)
    bnb = nc.declare_dram_parameter("bn_bias", [F], f32, isOutput=False)
    iden = nc.declare_dram_parameter("iden", [P, P], bf16, isOutput=False)
    y = nc.declare_dram_parameter("y", [F, F], f32, isOutput=True)

    AX = mybir.AxisListType.X
    OP = mybir.AluOpType
    ACT = mybir.ActivationFunctionType
    # prelu(v, w) = max(w*v, v) for w <= 1, min otherwise
    prelu_op = OP.max if w1 <= 1.0 else OP.min

    with tile.TileContext(nc) as tc:
        with (
            tc.tile_pool(name="dram", bufs=1, space="DRAM") as dpool,
            tc.tile_pool(name="xin", bufs=3) as xpool,
            tc.tile_pool(name="small", bufs=1) as spool,
            tc.tile_pool(name="acc", bufs=4) as apool,
            tc.tile_pool(name="mask", bufs=1) as mpool,
            tc.tile_pool(name="yout", bufs=4) as ypool,
        ):
            # threshold tile for the fused compare (in1 of scalar_tensor_tensor)
            thr = spool.tile([P, F], f32, tag="thr")
            nc.gpsimd.memset(thr[:], THRESH)
            idt = spool.tile([P, P], bf16, tag="idt")
            nc.gpsimd.dma_start(idt[:], iden[:])
            bsig = spool.tile([P, 1], f32, tag="bsig")
            nc.vector.memset(bsig[:], -THRESH * K_SAT - SAT_MARGIN)
            epsb = spool.tile([P, 1], f32, tag="epsb")
            nc.vector.memset(epsb[:], BN_EPS)
            bnw_t = spool.tile([P, NJ], f32, tag="bnw_t")
            nc.gpsimd.dma_start(bnw_t[:], bnw[:].rearrange("(j p) -> p j", p=P))
            bnb_t = spool.tile([P, NJ], f32, tag="bnb_t")
            nc.gpsimd.dma_start(bnb_t[:], bnb[:].rearrange("(j p) -> p j", p=P))

            # ---------- phase A: h[f] = mean_L prelu(x[f, :]) ----------
            # 1 MiB slab loads: chunks (2s, 2s+1) land side by side in one
            # [128, 2L] tile (fewer DMA issues); the fused prelu+row-sum then
            # runs once per chunk on the column halves.
            hsb = spool.tile([P, NJ], f32, tag="hsb")
            xv = x[:].rearrange("(s h p) l -> s p h l", p=P, h=2)
            for s in range(NJ // 2):
                xt = xpool.tile([P, 2 * L], f32, tag="x")
                dma = nc.sync.dma_start if s % 2 == 0 else nc.gpsimd.dma_start
                dma(xt[:].rearrange("p (h l) -> p h l", h=2), xv[s])
                for hh in range(2):
                    j = 2 * s + hh
                    xs = xt[:, hh * L : (hh + 1) * L]
                    # xs = prelu(xs) in place, hsb col = row-sum(prelu(xs)).
                    # hsb carries raw sums; the 1/L (2^-10, exact) is folded
                    # into phase B's scales, so results stay bit-identical.
                    nc.vector.scalar_tensor_tensor(
                        xs,
                        xs,
                        w1,
                        xs,
                        op0=OP.mult,
                        op1=prelu_op,
                        accum_out=hsb[:, j : j + 1],
                    )

            # ---------- phase B: BN batch stats via AllGather ----------
            # hd holds h in p-major order (hd[p*NJ + j] = h[j*128 + p]) so the
            # post-gather Hp load reads contiguous 64B runs instead of 4B
            # elements; only this vector's internal order changes.
            hd = dpool.tile([F], f32, tag="hd")
            Hd = dpool.tile([B, F], f32, tag="Hd")
            nc.sync.dma_start(hd[:].rearrange("(p j) -> p j", j=NJ), hsb[:])
            if num_devices > 1:
                nc.gpsimd.collective_compute(
                    "AllGather",
                    OP.bypass,
                    replica_groups=[list(range(B))],
                    ins=[hd[:].opt()],
                    outs=[Hd[:].opt()],
                )
            else:  # single-core timing model variant: fake the gather locally
                for b in range(B):
                    nc.sync.dma_start(
                        Hd[b : b + 1, :], hd[:].rearrange("(o f) -> o f", o=1)
                    )
            # Hp[p, b*NJ + j] = Hd[b, p*NJ+j] — one balanced 3-dim DMA with
            # 64B contiguous runs (j innermost on both sides)
            Hp = spool.tile([P, B * NJ], f32, tag="Hp")
            nc.sync.dma_start(
                Hp[:].rearrange("p (b j) -> p b j", j=NJ),
                Hd[:].rearrange("b (p j) -> p b j", j=NJ),
            )
            Hp3 = Hp[:].rearrange("p (b j) -> p j b", j=NJ)
            smu = spool.tile([P, NJ], f32, tag="smu")
            nc.vector.tensor_reduce(smu[:], Hp3, axis=AX, op=OP.add)
            mu = spool.tile([P, NJ], f32, tag="mu")
            nc.vector.tensor_scalar(mu[:], smu[:], 1.0 / B, None, op0=OP.mult)
            # two-pass variance: hcall = Hp - mu (mu free-dim-broadcast over b)
            mu3b = mu[:].rearrange("p (o j) -> p o j", o=1).to_broadcast([P, B, NJ])
            hcall = spool.tile([P, B * NJ], f32, tag="hcall")
            nc.vector.tensor_sub(
                hcall[:].rearrange("p (b j) -> p b j", j=NJ),
                Hp[:].rearrange("p (b j) -> p b j", j=NJ),
                mu3b,
            )
            hsq = spool.tile([P, B * NJ], f32, tag="hsq")
            nc.vector.tensor_mul(hsq[:], hcall[:], hcall[:])
            ssq = spool.tile([P, NJ], f32, tag="ssq")
            nc.vector.tensor_reduce(
                ssq[:], hsq[:].rearrange("p (b j) -> p j b", j=NJ), axis=AX, op=OP.add
            )
            # std = sqrt(ssq/(B*L^2) + eps): everything here is in the raw-sum
            # domain (hsb = L*h), so fold the exact 2^-23 into the sqrt scale
            stdt = spool.tile([P, NJ], f32, tag="stdt")
            nc.scalar.activation(
                stdt[:], ssq[:], ACT.Sqrt, bias=epsb[:], scale=1.0 / (B * L * L)
            )
            inv = spool.tile([P, NJ], f32, tag="inv")
            nc.vector.reciprocal(inv[:], stdt[:])
            # invL = inv/L turns sum-domain deviations back into mean-domain
            invL = spool.tile([P, NJ], f32, tag="invL")
            nc.vector.tensor_scalar(invL[:], inv[:], 1.0 / L, None, op0=OP.mult)
            # hh2 = ((h - mu) * inv) * bnw + bnb, matching the reference's order
            hc = spool.tile([P, NJ], f32, tag="hc")
            nc.vector.tensor_sub(hc[:], hsb[:], mu[:])
            hn = spool.tile([P, NJ], f32, tag="hn")
            nc.vector.tensor_mul(hn[:], hc[:], invL[:])
            hw = spool.tile([P, NJ], f32, tag="hw")
            nc.vector.tensor_mul(hw[:], hn[:], bnw_t[:])
            hh2 = spool.tile([P, NJ], f32, tag="hh2")
            nc.vector.tensor_add(hh2[:], hw[:], bnb_t[:])
            hhd = dpool.tile([F], f32, tag="hhd")
            nc.sync.dma_start(hhd[:].rearrange("(j p) -> p j", p=P), hh2[:])
            # hb[p, m] = hhat[m] for every partition p (broadcast row)
            hb = spool.tile([P, F], f32, tag="hb")
            nc.sync.dma_start(
                hb[:], hhd[:].rearrange("(o f) -> o f", o=1).to_broadcast([P, F])
            )

            # ---------- phase C: mask + degrees ----------
            # khh = K_SAT * hhat for the saturated-sigmoid scale
            khh = spool.tile([P, NJ], f32, tag="khh")
            nc.vector.tensor_scalar(khh[:], hh2[:], K_SAT, None, op0=OP.mult)
            dsb = spool.tile([P, NJ], f32, tag="dsb")
            masks = []
            for i in range(NJ):
                mt = mpool.tile([P, F], bf16, tag=f"m{i}", name=f"mask{i}")
                if i in ACT_CHUNKS:
                    # mask = sigmoid(K*(h_m*h_n - thresh) - margin) in {0,1}
                    nc.scalar.activation(
                        mt[:],
                        hb[:],
                        ACT.Sigmoid,
                        bias=bsig[:],
                        scale=khh[:, i : i + 1],
                        accum_out=dsb[:, i : i + 1],
                    )
                else:
                    # mask = (hb * h_n) > thr, degree = row-sum(mask)
                    nc.vector.scalar_tensor_tensor(
                        mt[:],
                        hb[:],
                        hh2[:, i : i + 1],
                        thr[:],
                        op0=OP.mult,
                        op1=OP.is_gt,
                        accum_out=dsb[:, i : i + 1],
                    )
                # add identity on the diagonal block (GpSimd: DVE/ACT are the
                # phase-C bottleneck, POOL is idle)
                nc.gpsimd.tensor_add(
                    mt[:, i * P : (i + 1) * P], mt[:, i * P : (i + 1) * P], idt[:]
                )
                masks.append(mt)

            # ---------- phase D: c = rsqrt(d + 1) ----------
            sqd = spool.tile([P, NJ], f32, tag="sqd")
            nc.scalar.activation(sqd[:], dsb[:], ACT.Sqrt, bias=1.0)
            csb = spool.tile([P, NJ], f32, tag="csb")
            nc.vector.reciprocal(csb[:], sqd[:])
            cd = dpool.tile([F], f32, tag="cd")
            nc.sync.dma_start(cd[:].rearrange("(j p) -> p j", p=P), csb[:])
            cbt = spool.tile([P, F], f32, tag="cbt")
            nc.sync.dma_start(
                cbt[:], cd[:].rearrange("(o f) -> o f", o=1).to_broadcast([P, F])
            )

            # ---------- phase E: y = mask * c_n * c_m ----------
            for i in range(NJ):
                yt = ypool.tile([P, F], f32, tag="yt")
                nc.vector.scalar_tensor_tensor(
                    yt[:],
                    cbt[:],
                    csb[:, i : i + 1],
                    masks[i][:],
                    op0=OP.mult,
                    op1=OP.mult,
                )
                # alternate DMA queues so descriptor issue overlaps transfer
                dma = nc.sync.dma_start if i % 2 == 0 else nc.gpsimd.dma_start
                dma(y[i * P : (i + 1) * P, :], yt[:])

    nc.compile()
    return nc


def _get_nc(w1: float):
    key = round(w1, 9)
    if key not in _NC_CACHE:
        _NC_CACHE[key] = _build(w1)
    return _NC_CACHE[key]


def _in_maps(x, bn_weight, bn_bias):
    iden = np.eye(P, dtype=mybir.dt.np(bf16))
    bnw = np.ascontiguousarray(np.asarray(bn_weight, dtype=np.float32))
    bnb = np.ascontiguousarray(np.asarray(bn_bias, dtype=np.float32))
    return [
        {
            "x": np.ascontiguousarray(np.asarray(x[b], dtype=np.float32)),
            "bn_weight": bnw,
            "bn_bias": bnb,
            "iden": iden,
        }
        for b in range(B)
    ]


def kernel(x, prelu1_w, prelu2_w, bn_weight, bn_bias):
    # prelu2 is the identity on the (non-negative) normalized adjacency.
    w1 = float(np.asarray(prelu1_w).reshape(-1)[0])
    nc = _get_nc(w1)
    res = run_bass_kernel_spmd(nc, _in_maps(x, bn_weight, bn_bias), list(range(B)))
    return np.stack([res.results[b]["y"] for b in range(B)]).astype(np.float32)


def kernel_traced(x, prelu1_w, prelu2_w, bn_weight, bn_bias, **trace_kwargs):
    """Same as kernel() but requesting NTFF tracing (if the env supports it)."""
    w1 = float(np.asarray(prelu1_w).reshape(-1)[0])
    nc = _get_nc(w1)
    res = run_bass_kernel_spmd(
        nc, _in_maps(x, bn_weight, bn_bias), list(range(B)), trace=True, **trace_kwargs
    )
    out = np.stack([res.results[b]["y"] for b in range(B)]).astype(np.float32)
    return out, res
